# revision 1
# baseline (speedup 1.0000x reference)
"""FastRNN Trainium2 kernel v2: transposed z-space recurrence, fp16 matmuls.

h_t = sigmoid(beta)*h_{t-1} + sigmoid(alpha)*tanh(x_t@W + h_{t-1}@U + b).

Strategy (data-parallel over batch, 8 NeuronCores, 8 sequences each):
  - Substitute h = sa*H (sa = sigmoid(alpha)): H_n+1 = sb*H_n + c_n with
    c_n = tanh(wx_n + b + Ua H_n), Ua = sa*U. Output h = sa*H is a host-side
    scalar scale.
  - z-space: z_n = Ua H_n obeys z_n = sb*z_{n-1} + Ua c_{n-1}, so the PSUM
    accumulation per step is psum_n = wxb_n + sb*z_{n-1} + Ua c_{n-1}: the
    matmul moving operands are the PREVIOUS tanh output c16 and zsb16 =
    fp16(sb*(psum - wxb)) directly -- the critical loop is PE -> tanh -> PE
    with the two DVE ops (zsb16 extraction, H-history update) off the loop.
  - Everything transposed: [128(h%128), k(h//128), b] tiles; one batched
    tanh [128,32] and two batched DVE ops per step; no PE transposes.
  - U as 16 fp16 128x128 stationary blocks (FWL on HW); wx/z/c injected via
    a single fp16 identity stationary shared by 8 tiny matmuls per step.
  - wxb precomputed via fp32r matmuls (W stationary, pre-transposed x moving
    N=512); bias folded into wxb via ACT Identity's per-partition bias port.
"""

from contextlib import ExitStack

import numpy as np

import concourse.bass as bass
import concourse.mybir as mybir
from concourse.bass_utils import run_bass_kernel_spmd
from concourse.tile import TileContext
from concourse.vector_clock import ScopedClock

F32 = mybir.dt.float32
F32R = mybir.dt.float32r
F16 = mybir.dt.float16
AF = mybir.ActivationFunctionType
ALU = mybir.AluOpType

B, T, I, H = 64, 512, 256, 512
N_CORES = 8
BC = B // N_CORES
KT = H // 128
KI = I // 128
TT = T // 16


class PatchedTileContext(TileContext):
    """The stock tail drain can carry more sem waits than this walrus's
    CTRL-instruction wait slots; spill the excess onto preceding NOPs."""

    def _drain_and_barrier(self, tick_clock, wait_clock):
        nc = self.nc
        drain_inst = nc.sync.drain()
        wait_clock.add_sem_waits(
            drain_inst.ins, ScopedClock({None: tick_clock.global_clock})
        )
        si = drain_inst.ins.sync_info
        waits = list(si.on_wait or []) if si is not None else []
        if len(waits) > 1:
            bb = nc.cur_bb.bb
            idx = bb.instructions.index(drain_inst.ins)
            extra, keep = waits[:-1], waits[-1:]
            si.on_wait = keep
            for i in range(len(extra)):
                nop = nc.sync.nop()
                nsi = nop.ins.sync_info
                if nsi is None:
                    nop.ins.sync_info = mybir.SyncInfo(
                        on_wait=extra[i : i + 1], on_update=[]
                    )
                else:
                    nsi.on_wait = extra[i : i + 1]
                bb.instructions.remove(nop.ins)
                bb.instructions.insert(idx, nop.ins)
                idx += 1
        nc.all_engine_barrier()
        popped = nc._tile_sem_poison_stack.pop()
        assert popped is self._sem_poison
        nc.clear_and_free_semaphores(list(self.sems.allocated().values()))
        nc.all_engine_barrier()


_CTRL_TYPES = ("InstDrain", "InstNop", "InstEventSemOp")


def spill_waits(nc, compute_limit=1, ctrl_limit=1):
    """Move excess per-instruction sync waits onto preceding same-engine NOPs
    (this walrus accepts at most one wait slot per instruction)."""
    for f in nc.m.functions:
        for bb in f.blocks:
            insts = list(bb.instructions)
            for inst in insts:
                si = inst.sync_info
                if si is None or not si.on_wait:
                    continue
                limit = (
                    ctrl_limit
                    if type(inst).__name__ in _CTRL_TYPES
                    else compute_limit
                )
                waits = list(si.on_wait)
                if len(waits) <= limit:
                    continue
                keep = waits[-limit:]
                extra = waits[:-limit]
                si.on_wait = keep
                idx = bb.instructions.index(inst)
                for i in range(0, len(extra), ctrl_limit):
                    nop = nc.engines[inst.engine].nop()
                    nsi = nop.ins.sync_info
                    chunk = extra[i : i + ctrl_limit]
                    if nsi is None:
                        nop.ins.sync_info = mybir.SyncInfo(
                            on_wait=chunk, on_update=[]
                        )
                    else:
                        nsi.on_wait = chunk
                    for f2 in nc.m.functions:
                        for bb2 in f2.blocks:
                            if nop.ins in bb2.instructions:
                                bb2.instructions.remove(nop.ins)
                    bb.instructions.insert(idx, nop.ins)
                    idx += 1


def build_nc(sb: float):
    nc = bass.Bass(
        "TRN2", target_bir_lowering=False, debug=False, num_devices=N_CORES
    )

    xt = nc.dram_tensor("xt", [I, BC * T], F32, kind="ExternalInput")
    w = nc.dram_tensor("w", [I, H], F32, kind="ExternalInput")
    u16 = nc.dram_tensor("u16", [128, KT * KT * 128], F16, kind="ExternalInput")
    biasT = nc.dram_tensor("biasT", [128, KT], F32, kind="ExternalInput")
    id16 = nc.dram_tensor("id16", [128, 128], F16, kind="ExternalInput")
    out = nc.dram_tensor("out", [128, KT, T, BC], F16, kind="ExternalOutput")

    with PatchedTileContext(nc) as tc, ExitStack() as ctx:
        pool = lambda **kw: ctx.enter_context(tc.tile_pool(**kw))
        const = pool(name="const", bufs=1)
        u_sb = const.tile([128, KT, KT, 128], F16)
        w_sb = const.tile([128, KI, H], F32R)
        biasT_sb = const.tile([128, KT], F32)
        id16_sb = const.tile([128, 128], F16)
        xt_sb = const.tile([128, KI, BC * T], F32R)
        wxb16 = const.tile([128, KT, T, BC], F16)
        wxbsb16 = const.tile([128, KT, T, BC], F16)
        hist = const.tile([128, KT, T + 1, BC], F16)

        nc.gpsimd.dma_start(
            out=u_sb[:], in_=u16[:].rearrange("p (kk k f) -> p kk k f", kk=KT, k=KT)
        )
        for j in range(KI):
            nc.gpsimd.dma_start(
                out=w_sb[:, j, :], in_=w[j * 128 : (j + 1) * 128, :]
            )
        nc.sync.dma_start(out=biasT_sb[:], in_=biasT[:])
        nc.sync.dma_start(out=id16_sb[:], in_=id16[:])
        nc.vector.memset(hist[:, :, 0, :], 0.0)
        for j in range(KI):
            nc.gpsimd.dma_start(
                out=xt_sb[:, j, :], in_=xt[j * 128 : (j + 1) * 128, :]
            )

        # ---- phase 1: wxb16[p, k, t, b] = fp16(sum_i W[i,128k+p] x[b,t,i] + bias)
        # tokens are b-major in xt_sb (seq b occupies cols [b*T, (b+1)*T)).
        ps1 = pool(name="ps1", bufs=4, space="PSUM")
        for k in range(KT):
            for half in range(2):
                tiles = []
                for b4 in range(4):
                    b = half * 4 + b4
                    ps = ps1.tile([128, T], F32, tag="p1")
                    tiles.append((b, ps))
                for j in range(KI):
                    for b, ps in tiles:
                        nc.tensor.matmul(
                            ps[:],
                            w_sb[:, j, k * 128 : (k + 1) * 128],
                            xt_sb[:, j, b * T : (b + 1) * T],
                            start=(j == 0),
                            stop=(j == KI - 1),
                        )
                for b, ps in tiles:
                    nc.scalar.activation(
                        wxb16[:, k, :, b], ps[:], AF.Identity,
                        bias=biasT_sb[:, k : k + 1],
                    )
                    nc.vector.tensor_scalar_mul(
                        wxbsb16[:, k, :, b], wxb16[:, k, :, b], float(sb)
                    )

        # ---- recurrence (z-space) ----
        psum = pool(name="ps2", bufs=2, space="PSUM")
        cp = pool(name="c", bufs=3)
        zp = pool(name="z", bufs=3)
        c_prev = None
        z_prev = None
        for n in range(T):
            pp = psum.tile([128, KT, BC], F32, tag="mm")
            first = n == 0
            # start=True clears the whole PSUM bank's has_written bits, so
            # the start (and stop) matmuls must cover the full tile.
            nc.tensor.matmul(
                pp[:], id16_sb[:], wxb16[:, :, n, :], start=True, stop=first
            )
            if not first:
                for kk in range(KT):
                    for k in range(KT):
                        nc.tensor.matmul(
                            pp[:, k, :],
                            u_sb[:, kk, k, :],
                            c_prev[:, kk, :],
                            start=False,
                            stop=False,
                            skip_group_check=True,
                        )
                nc.tensor.matmul(
                    pp[:], id16_sb[:], z_prev[:], start=False, stop=True,
                    skip_group_check=True,
                )
            c_cur = cp.tile([128, KT, BC], F16)
            nc.scalar.activation(c_cur[:], pp[:], AF.Tanh)
            z_cur = zp.tile([128, KT, BC], F16)
            nc.vector.scalar_tensor_tensor(
                out=z_cur[:], in0=pp[:], scalar=float(sb),
                in1=wxbsb16[:, :, n, :], op0=ALU.mult, op1=ALU.subtract,
            )
            nc.vector.scalar_tensor_tensor(
                out=hist[:, :, n + 1, :], in0=hist[:, :, n, :],
                scalar=float(sb), in1=c_cur[:], op0=ALU.mult, op1=ALU.add,
            )
            c_prev, z_prev = c_cur, z_cur

        # ---- output DMA (fp16 transposed history; host permutes+scales) ----
        for cch in range(TT):
            t0 = cch * 16
            nc.sync.dma_start(
                out=out[:, :, t0 : t0 + 16, :],
                in_=hist[:, :, 1 + t0 : 1 + t0 + 16, :],
            )

    spill_waits(nc, compute_limit=1)
    return nc


_CACHE = {}


def kernel(x, W, U, bias, alpha, beta):
    x = np.asarray(x, np.float32)
    W = np.ascontiguousarray(np.asarray(W, np.float32))
    U = np.asarray(U, np.float32)
    bias = np.asarray(bias, np.float32)
    sa = float(1.0 / (1.0 + np.exp(-np.float64(np.asarray(alpha).reshape(-1)[0]))))
    sb = float(1.0 / (1.0 + np.exp(-np.float64(np.asarray(beta).reshape(-1)[0]))))

    key = (sa, sb)
    if key not in _CACHE:
        _CACHE[key] = build_nc(sb)
    nc = _CACHE[key]

    u16 = np.ascontiguousarray(
        (sa * U).astype(np.float16)
        .reshape(KT, 128, KT, 128).transpose(1, 0, 2, 3).reshape(128, KT * KT * 128)
    )
    biasT = np.ascontiguousarray(bias.reshape(KT, 128).T)
    id16 = np.eye(128, dtype=np.float16)

    in_maps = []
    for c in range(N_CORES):
        xc = x[c * BC : (c + 1) * BC]  # [BC, T, I]
        xtc = np.ascontiguousarray(xc.transpose(2, 0, 1).reshape(I, BC * T))
        in_maps.append({
            "xt": xtc, "w": W, "u16": u16, "biasT": biasT, "id16": id16,
        })

    res = run_bass_kernel_spmd(nc, in_maps, list(range(N_CORES))).results

    out = np.empty((B, T, H), np.float32)
    for c in range(N_CORES):
        # out_raw[p, k, t, b] = H_t[b, 128k+p] -> h = sa*H, layout permute
        out[c * BC : (c + 1) * BC] = (
            res[c]["out"].astype(np.float32).transpose(3, 1, 0, 2)
            .reshape(BC, H, T).transpose(0, 2, 1)
        ) * np.float32(sa)
    return out

